# revision 7
# baseline (speedup 1.0000x reference)
"""Trainium2 Bass kernel for nn_LinearTemporalSelfAttention (B=4,T=8192,D=512,H=8).

Sharding: 8 cores = B(4) x T-halves(2). Each core owns a (b, t-half) slab
(4096 x 512) end-to-end. Cross-core data is only the KV-state einsum
(sum over full T) and the emb projection (emb_W sharded over TE within a
pair) — both folded into ONE pair-wise AllReduce of a 134 KB buffer.

Math notes (exact up to fp assoc):
 - softmax shift-invariance: exp(k)/sum(exp(k)) without max-subtraction
   (values are O(1) after LN with 0.02-scale weights).
 - k-mask (+ -1e6) is replaced by masking v (v*mask) and computing the
   softmax-T denominator S = sum_t exp(k)*mask via an extra column of
   ones*mask appended to v in the same PE matmul.
 - gamma/beta of LN1 are folded into Wq/Wk/Wv (+ biases) on the host.
 - attn normalization (1/S) is applied to the tiny (64,8,65) state, and
   the q-softmax denominator (1/sum) is applied to y after the q@attn
   matmul — so the big T-sized tensors never need normalizing passes.
"""
import numpy as np
import ml_dtypes

B, T, D, H, TE = 4, 8192, 512, 8, 2048
Dh = D // H          # 64
EPS = 1e-5
NCORES = 8
TH = T // 2          # 4096 rows per core
P = 128
NT = TH // P         # 32 row tiles
KC = D // P          # 4 contraction chunks
TEH = TE // 2        # 1024 te rows per core
TEC = TEH // P       # 8 te chunks
CCU = 64 * H * (Dh + 1)     # 33280 floats of U_aug
CCN = CCU + 2 * D           # + emb partial

_CACHE: dict = {}


def _build(flags):
    has_bq, has_bk, has_bv, has_outb, has_embb = flags
    from contextlib import ExitStack
    import concourse.bass as bass
    import concourse.bacc as bacc
    import concourse.tile as tile
    import concourse.mybir as mybir
    from concourse.masks import make_identity

    f32 = mybir.dt.float32
    bf16 = mybir.dt.bfloat16
    Alu = mybir.AluOpType
    Act = mybir.ActivationFunctionType

    nc = bacc.Bacc("TRN2", target_bir_lowering=False, debug=False,
                   enable_asserts=True, num_devices=NCORES)

    x_in = nc.declare_dram_parameter("x", [TH, D], f32, isOutput=False)
    mk_in = nc.declare_dram_parameter("mask", [TH], f32, isOutput=False)
    emb_in = nc.declare_dram_parameter("embv", [TEH], f32, isOutput=False)
    wq_in = nc.declare_dram_parameter("wq", [KC, P, D], bf16, isOutput=False)
    wk_in = nc.declare_dram_parameter("wk", [KC, P, D], bf16, isOutput=False)
    wv_in = nc.declare_dram_parameter("wv", [KC, P, D], bf16, isOutput=False)
    wo_in = nc.declare_dram_parameter("wo", [KC, P, D], bf16, isOutput=False)
    we_in = nc.declare_dram_parameter("we", [TEC, P, 2 * D], bf16, isOutput=False)
    vec_in = nc.declare_dram_parameter("vecs", [1, 8, D], f32, isOutput=False)
    y_out = nc.declare_dram_parameter("y", [TH, D], f32, isOutput=True)

    PAIRS = [[0, 1], [2, 3], [4, 5], [6, 7]]

    with tile.TileContext(nc) as tc, ExitStack() as ctx:
        const = ctx.enter_context(tc.tile_pool(name="const", bufs=1))
        wpool = ctx.enter_context(tc.tile_pool(name="wpool", bufs=1))
        xstash = ctx.enter_context(tc.tile_pool(name="xstash", bufs=NT))
        qstash = ctx.enter_context(tc.tile_pool(name="qstash", bufs=NT))
        dramp = ctx.enter_context(tc.tile_pool(name="dram", bufs=1, space="DRAM"))

        ident = const.tile([P, P], bf16)
        make_identity(nc, ident)
        eps_t = const.tile([P, 1], f32)
        nc.vector.memset(eps_t, EPS)
        ones8 = const.tile([P, H, 1], bf16)
        nc.vector.memset(ones8, 1.0)
        ones_row = const.tile([1, P], bf16)
        nc.vector.memset(ones_row, 1.0)

        wq_s = wpool.tile([P, KC, D], bf16)
        nc.sync.dma_start(out=wq_s, in_=wq_in[:].rearrange("c p d -> p c d"))
        wk_s = wpool.tile([P, KC, D], bf16)
        nc.sync.dma_start(out=wk_s, in_=wk_in[:].rearrange("c p d -> p c d"))
        wv_s = wpool.tile([P, KC, D], bf16)
        nc.sync.dma_start(out=wv_s, in_=wv_in[:].rearrange("c p d -> p c d"))
        wo_s = wpool.tile([P, KC, D], bf16)
        nc.sync.dma_start(out=wo_s, in_=wo_in[:].rearrange("c p d -> p c d"))
        we_s = wpool.tile([P, TEC, 2 * D], bf16)
        nc.sync.dma_start(out=we_s, in_=we_in[:].rearrange("c p d -> p c d"))
        mask_s = wpool.tile([P, NT], f32)
        nc.sync.dma_start(out=mask_s, in_=mk_in[:].rearrange("(n p) -> p n", p=P))
        vec_s = wpool.tile([1, 8, D], f32)
        nc.sync.dma_start(out=vec_s, in_=vec_in[:])
        rq_all = wpool.tile([P, NT, H], f32)

        cc_in_t = dramp.tile([CCN], f32)
        cc_out_t = dramp.tile([CCN], f32)

        x_tiles = []
        q_tiles = []

        with ExitStack() as ctxA:
            work = ctxA.enter_context(tc.tile_pool(name="work", bufs=3))
            psA = ctxA.enter_context(tc.tile_pool(name="psA", bufs=1, space="PSUM"))
            psT = ctxA.enter_context(tc.tile_pool(name="psT", bufs=2, space="PSUM"))
            psU = ctxA.enter_context(tc.tile_pool(name="psU", bufs=1, space="PSUM"))
            embp = ctxA.enter_context(tc.tile_pool(name="embp", bufs=1))

            # ---- bias broadcast tiles (only when biases nonzero) ----
            # broadcast row -> [P, D] via PE: ones[1,P].T @ row[1,D]
            def bcast_row(row_idx, name):
                pb = psT.tile([P, D], f32, tag="pT")
                rbf = const.tile([1, D], bf16, tag="rbf_" + name)
                nc.vector.tensor_copy(out=rbf, in_=vec_s[:, row_idx, :])
                nc.tensor.matmul(out=pb, lhsT=ones_row, rhs=rbf,
                                 start=True, stop=True)
                bc = const.tile([P, D], f32, tag="bc_" + name)
                nc.scalar.copy(out=bc, in_=pb)
                return bc

            bq_bc = bcast_row(0, "bq") if has_bq else None
            bk_bc = bcast_row(1, "bk") if has_bk else None
            bv_bc = bcast_row(2, "bv") if has_bv else None
            ob_bc = bcast_row(3, "ob") if has_outb else None

            # ---- emb projection partial (this core's TE shard) ----
            embt = embp.tile([P, TEC], f32)
            nc.sync.dma_start(out=embt, in_=emb_in[:].rearrange("(c p) -> p c", p=P))
            embsg = embp.tile([P, TEC], f32)
            nc.scalar.activation(out=embsg, in_=embt, func=Act.Sigmoid)
            embs = embp.tile([P, TEC], bf16)
            nc.vector.tensor_mul(out=embs, in0=embt, in1=embsg)
            pe0 = psA.tile([1, D], f32, tag="pq")
            pe1 = psA.tile([1, D], f32, tag="pk")
            for j in range(TEC):
                nc.tensor.matmul(out=pe0, lhsT=embs[:, j:j + 1],
                                 rhs=we_s[:, j, 0:D],
                                 start=(j == 0), stop=(j == TEC - 1))
            for j in range(TEC):
                nc.tensor.matmul(out=pe1, lhsT=embs[:, j:j + 1],
                                 rhs=we_s[:, j, D:2 * D],
                                 start=(j == 0), stop=(j == TEC - 1))
            emb_part = embp.tile([1, 2 * D], f32)
            nc.scalar.copy(out=emb_part[:, 0:D], in_=pe0)
            nc.scalar.copy(out=emb_part[:, D:2 * D], in_=pe1)

            u0 = psU.tile([64, 4, Dh + 1], f32, tag="u0")
            u1 = psU.tile([64, 4, Dh + 1], f32, tag="u1")

            # ---- phase A: LN, QKV projections, exp, U accumulation ----
            for i in range(NT):
                xt = xstash.tile([P, D], f32, tag="x")
                x_tiles.append(xt)
                nc.sync.dma_start(out=xt, in_=x_in[i * P:(i + 1) * P, :])
                st = work.tile([P, 6], f32, tag="st")
                nc.vector.bn_stats(out=st, in_=xt)
                mv = work.tile([P, 2], f32, tag="mv")
                nc.vector.bn_aggr(out=mv, in_=st)
                sd = work.tile([P, 1], f32, tag="sd")
                nc.scalar.activation(out=sd, in_=mv[:, 1:2], func=Act.Sqrt,
                                     bias=eps_t)
                rstd = work.tile([P, 1], f32, tag="rstd")
                nc.vector.reciprocal(out=rstd, in_=sd)
                xn = work.tile([P, D], bf16, tag="xn")
                nc.vector.tensor_scalar(out=xn, in0=xt, scalar1=mv[:, 0:1],
                                        scalar2=rstd, op0=Alu.subtract,
                                        op1=Alu.mult)
                pT = psT.tile([P, KC, P], bf16, tag="pT")
                for j in range(KC):
                    nc.tensor.transpose(out=pT[:, j, :],
                                        in_=xn[:, j * P:(j + 1) * P],
                                        identity=ident)
                xT = work.tile([P, KC, P], bf16, tag="xT")
                nc.scalar.copy(out=xT, in_=pT)

                pq = psA.tile([P, D], f32, tag="pq")
                pk = psA.tile([P, D], f32, tag="pk")
                pv = psA.tile([P, D], f32, tag="pv")
                for j in range(KC):
                    nc.tensor.matmul(out=pq, lhsT=xT[:, j, :], rhs=wq_s[:, j, :],
                                     start=(j == 0), stop=(j == KC - 1))
                    nc.tensor.matmul(out=pk, lhsT=xT[:, j, :], rhs=wk_s[:, j, :],
                                     start=(j == 0), stop=(j == KC - 1))
                    nc.tensor.matmul(out=pv, lhsT=xT[:, j, :], rhs=wv_s[:, j, :],
                                     start=(j == 0), stop=(j == KC - 1))
                if has_bq:
                    nc.vector.tensor_add(out=pq, in0=pq, in1=bq_bc)
                if has_bk:
                    nc.vector.tensor_add(out=pk, in0=pk, in1=bk_bc)
                if has_bv:
                    nc.vector.tensor_add(out=pv, in0=pv, in1=bv_bc)

                qt = qstash.tile([P, D], bf16, tag="qt")
                q_tiles.append(qt)
                qs = work.tile([P, H], f32, tag="qs")
                for h in range(H):
                    nc.scalar.activation(out=qt[:, h * Dh:(h + 1) * Dh],
                                         in_=pq[:, h * Dh:(h + 1) * Dh],
                                         func=Act.Exp,
                                         accum_out=qs[:, h:h + 1])
                nc.vector.reciprocal(out=rq_all[:, i, :], in_=qs)

                et = work.tile([P, D], bf16, tag="et")
                nc.scalar.activation(out=et, in_=pk, func=Act.Exp)

                va = work.tile([P, H, Dh + 1], bf16, tag="va")
                nc.vector.tensor_scalar_mul(
                    out=va[:, :, 0:Dh],
                    in0=pv[:].rearrange("p (h d) -> p h d", h=H),
                    scalar1=mask_s[:, i:i + 1])
                nc.vector.tensor_scalar_mul(out=va[:, :, Dh:Dh + 1], in0=ones8,
                                            scalar1=mask_s[:, i:i + 1])
                for h in range(H):
                    u = u0 if h < 4 else u1
                    # one accumulation group per PSUM bank: start clears the
                    # whole zero-region once; has_written bits make the first
                    # write to each head slot an overwrite, later ones adds.
                    nc.tensor.matmul(out=u[:, h % 4, :],
                                     lhsT=et[:, h * Dh:(h + 1) * Dh],
                                     rhs=va[:, h, :],
                                     start=(i == 0 and h % 4 == 0),
                                     stop=(i == NT - 1 and h % 4 == 3))

            # ---- ship partials through the pair AllReduce ----
            u_sb = embp.tile([64, H, Dh + 1], f32)
            nc.scalar.copy(out=u_sb[:, 0:4, :], in_=u0)
            nc.scalar.copy(out=u_sb[:, 4:8, :], in_=u1)
            nc.sync.dma_start(
                out=cc_in_t[0:CCU].rearrange("(p h f) -> p h f", p=64, h=H),
                in_=u_sb)
            nc.sync.dma_start(
                out=cc_in_t[CCU:CCN].rearrange("(a f) -> a f", a=1),
                in_=emb_part)
            nc.gpsimd.collective_compute(
                "AllReduce", Alu.add, replica_groups=PAIRS,
                ins=[cc_in_t[:]], outs=[cc_out_t[:]])

        # ---- phase B prologue: attn state + stylization vectors ----
        with ExitStack() as ctxB:
            workB = ctxB.enter_context(tc.tile_pool(name="workB", bufs=3))
            psB = ctxB.enter_context(tc.tile_pool(name="psB", bufs=2, space="PSUM"))
            embB = ctxB.enter_context(tc.tile_pool(name="embB", bufs=1))

            u_f = embB.tile([64, H, Dh + 1], f32)
            nc.sync.dma_start(
                out=u_f, in_=cc_out_t[0:CCU].rearrange(
                    "(p h f) -> p h f", p=64, h=H))
            emb_f = embB.tile([1, 2 * D], f32)
            nc.sync.dma_start(
                out=emb_f, in_=cc_out_t[CCU:CCN].rearrange("(a f) -> a f", a=1))

            rs = embB.tile([64, H, 1], f32)
            nc.vector.reciprocal(out=rs, in_=u_f[:, :, Dh:Dh + 1])
            attn = embB.tile([64, H, Dh], bf16)
            for h in range(H):
                nc.vector.tensor_scalar_mul(out=attn[:, h, :],
                                            in0=u_f[:, h, 0:Dh],
                                            scalar1=rs[:, h, :])

            srow = embB.tile([1, D], f32)
            shrow = embB.tile([1, D], f32)
            if has_embb:
                nc.vector.tensor_add(out=srow, in0=emb_f[:, 0:D],
                                     in1=vec_s[:, 6, :])
                nc.vector.tensor_add(out=shrow, in0=emb_f[:, D:2 * D],
                                     in1=vec_s[:, 7, :])
            else:
                nc.vector.tensor_copy(out=srow, in_=emb_f[:, 0:D])
                nc.vector.tensor_copy(out=shrow, in_=emb_f[:, D:2 * D])
            t1 = embB.tile([1, D], f32)
            nc.vector.tensor_scalar_add(out=t1, in0=srow, scalar1=1.0)
            arow = embB.tile([1, D], bf16)
            nc.vector.tensor_mul(out=arow, in0=t1, in1=vec_s[:, 4, :])
            crow_f = embB.tile([1, D], f32)
            nc.vector.tensor_mul(out=crow_f, in0=t1, in1=vec_s[:, 5, :])
            nc.vector.tensor_add(out=crow_f, in0=crow_f, in1=shrow)
            crow = embB.tile([1, D], bf16)
            nc.vector.tensor_copy(out=crow, in_=crow_f)

            # broadcast a,c rows to [P, D] via PE ones-outer-product
            pa = psB.tile([P, D], f32, tag="py")
            nc.tensor.matmul(out=pa, lhsT=ones_row, rhs=arow,
                             start=True, stop=True)
            a_bc = embB.tile([P, D], f32)
            nc.scalar.copy(out=a_bc, in_=pa)
            pc = psB.tile([P, D], f32, tag="py")
            nc.tensor.matmul(out=pc, lhsT=ones_row, rhs=crow,
                             start=True, stop=True)
            c_bc = embB.tile([P, D], f32)
            nc.scalar.copy(out=c_bc, in_=pc)

            # ---- phase B: y = q@attn, LN2, stylize, silu, out proj ----
            for i in range(NT):
                qt = q_tiles[i]
                # per-head transpose keeps every head's q^T at base
                # partition 0 (PE rejects operands based at partition 64)
                pT2 = psB.tile([64, H, P], bf16, tag="pT2")
                for h in range(H):
                    nc.tensor.transpose(out=pT2[:, h, :],
                                        in_=qt[:, h * Dh:(h + 1) * Dh],
                                        identity=ident)
                qT = workB.tile([64, H, P], bf16, tag="qT")
                nc.scalar.copy(out=qT, in_=pT2)
                py = psB.tile([P, D], f32, tag="py")
                for h in range(H):
                    nc.tensor.matmul(
                        out=py[:, h * Dh:(h + 1) * Dh],
                        lhsT=qT[:, h, :],
                        rhs=attn[:, h, :],
                        start=True, stop=True)
                ysb = workB.tile([P, D], f32, tag="ysb")
                for h in range(H):
                    nc.vector.tensor_scalar_mul(
                        out=ysb[:, h * Dh:(h + 1) * Dh],
                        in0=py[:, h * Dh:(h + 1) * Dh],
                        scalar1=rq_all[:, i, h:h + 1])
                st2 = workB.tile([P, 6], f32, tag="st2")
                nc.vector.bn_stats(out=st2, in_=ysb)
                mv2 = workB.tile([P, 2], f32, tag="mv2")
                nc.vector.bn_aggr(out=mv2, in_=st2)
                sd2 = workB.tile([P, 1], f32, tag="sd2")
                nc.scalar.activation(out=sd2, in_=mv2[:, 1:2], func=Act.Sqrt,
                                     bias=eps_t)
                rstd2 = workB.tile([P, 1], f32, tag="rstd2")
                nc.vector.reciprocal(out=rstd2, in_=sd2)
                z2 = workB.tile([P, D], f32, tag="z2")
                nc.vector.tensor_scalar(out=z2, in0=ysb, scalar1=mv2[:, 0:1],
                                        scalar2=rstd2, op0=Alu.subtract,
                                        op1=Alu.mult)
                h1 = workB.tile([P, D], f32, tag="h1")
                nc.gpsimd.tensor_mul(out=h1, in0=z2, in1=a_bc)
                nc.gpsimd.tensor_add(out=h1, in0=h1, in1=c_bc)
                sg = workB.tile([P, D], bf16, tag="sg")
                nc.scalar.activation(out=sg, in_=h1, func=Act.Sigmoid)
                hs = workB.tile([P, D], bf16, tag="hs")
                nc.gpsimd.tensor_mul(out=hs, in0=h1, in1=sg)
                pT3 = psB.tile([P, KC, P], bf16, tag="pT3")
                for j in range(KC):
                    nc.tensor.transpose(out=pT3[:, j, :],
                                        in_=hs[:, j * P:(j + 1) * P],
                                        identity=ident)
                hT = workB.tile([P, KC, P], bf16, tag="hT")
                nc.scalar.copy(out=hT, in_=pT3)
                po = psB.tile([P, D], f32, tag="po")
                for j in range(KC):
                    nc.tensor.matmul(out=po, lhsT=hT[:, j, :],
                                     rhs=wo_s[:, j, :],
                                     start=(j == 0), stop=(j == KC - 1))
                osb = workB.tile([P, D], f32, tag="osb")
                nc.vector.tensor_add(out=osb, in0=po, in1=x_tiles[i])
                if has_outb:
                    nc.vector.tensor_add(out=osb, in0=osb, in1=ob_bc)
                nc.sync.dma_start(out=y_out[i * P:(i + 1) * P, :], in_=osb)

    nc.compile()
    return nc


def _prep(inputs, flags):
    bf = ml_dtypes.bfloat16
    x = np.asarray(inputs["x"], np.float32)
    emb = np.asarray(inputs["emb"], np.float32)
    src_mask = np.asarray(inputs["src_mask"], np.float32)
    gamma = np.asarray(inputs["gamma"], np.float32)
    beta = np.asarray(inputs["beta"], np.float32)
    gamma2 = np.asarray(inputs["gamma2"], np.float32)
    beta2 = np.asarray(inputs["beta2"], np.float32)
    emb_b = np.asarray(inputs["emb_b"], np.float32)
    out_b = np.asarray(inputs["out_b"], np.float32)

    def foldW(Wname):
        W = np.asarray(inputs[Wname], np.float32)
        return np.ascontiguousarray(
            (gamma[:, None] * W).astype(bf).reshape(KC, P, D))

    wq, wk, wv = foldW("Wq"), foldW("Wk"), foldW("Wv")
    wo = np.ascontiguousarray(
        np.asarray(inputs["out_W"], np.float32).astype(bf).reshape(KC, P, D))
    bq_f = np.asarray(inputs["bq"], np.float32) + beta @ np.asarray(inputs["Wq"], np.float32)
    bk_f = np.asarray(inputs["bk"], np.float32) + beta @ np.asarray(inputs["Wk"], np.float32)
    bv_f = np.asarray(inputs["bv"], np.float32) + beta @ np.asarray(inputs["Wv"], np.float32)
    vecs = np.ascontiguousarray(np.stack(
        [bq_f, bk_f, bv_f, out_b, gamma2, beta2, emb_b[:D], emb_b[D:]]
    ).astype(np.float32).reshape(1, 8, D))
    emb_W = np.asarray(inputs["emb_W"], np.float32)
    we_halves = [
        np.ascontiguousarray(
            emb_W[t * TEH:(t + 1) * TEH].astype(bf).reshape(TEC, P, 2 * D))
        for t in range(2)]

    in_maps = []
    for c in range(NCORES):
        b, th = c // 2, c % 2
        sl = slice(th * TH, (th + 1) * TH)
        in_maps.append({
            "x": np.ascontiguousarray(x[b, sl]),
            "mask": np.ascontiguousarray(src_mask[b, sl, 0]),
            "embv": np.ascontiguousarray(emb[b, th * TEH:(th + 1) * TEH]),
            "wq": wq, "wk": wk, "wv": wv, "wo": wo,
            "we": we_halves[th],
            "vecs": vecs,
        })
    return in_maps


def _flags(inputs):
    gamma = np.asarray(inputs["gamma"], np.float32)
    beta = np.asarray(inputs["beta"], np.float32)

    def nz(v):
        return bool(np.any(np.asarray(v) != 0))

    bq_f = np.asarray(inputs["bq"], np.float32) + beta @ np.asarray(inputs["Wq"], np.float32)
    bk_f = np.asarray(inputs["bk"], np.float32) + beta @ np.asarray(inputs["Wk"], np.float32)
    bv_f = np.asarray(inputs["bv"], np.float32) + beta @ np.asarray(inputs["Wv"], np.float32)
    return (nz(bq_f), nz(bk_f), nz(bv_f), nz(inputs["out_b"]), nz(inputs["emb_b"]))


def get_nc_and_inmaps(**inputs):
    flags = _flags(inputs)
    if flags not in _CACHE:
        _CACHE[flags] = _build(flags)
    return _CACHE[flags], _prep(inputs, flags)


def kernel(**inputs):
    from concourse.bass_utils import run_bass_kernel_spmd
    nc, in_maps = get_nc_and_inmaps(**inputs)
    res = run_bass_kernel_spmd(nc, in_maps, list(range(NCORES)))
    out = np.empty((B, T, D), np.float32)
    for c in range(NCORES):
        b, th = c // 2, c % 2
        out[b, th * TH:(th + 1) * TH] = res.results[c]["y"]
    return out
